# revision 1
# baseline (speedup 1.0000x reference)
"""Trainium2 Bass kernel for nn_Attention_7670811590880.

Multi-head attention prefill (B=1, S=2048, D=4096, H=32, KVH=8, HD=128),
tensor-parallel over heads across 8 NeuronCores.

Sharding: query head g uses kv head g % 8 (the reference's _repeat_kv inserts
the repeat axis BEFORE the kv-head axis). Core c takes query heads
{c, c+8, c+16, c+24} and kv head c; wo rows for those heads. Each core
produces a partial [S, D] output; the host sums the 8 partials.

On-device dataflow per core (all matmuls in float32r, TF32-class):
  Phase A  per s-block (512): stream xT d-chunks; 6 accumulating matmuls
           (4 qT heads, kT, vT in [head_dim, s] orientation); RoPE fused into
           the PSUM->SBUF move on DVE (head_dim rows permuted evens-then-odds
           via host-side wq/wk column permutation).
  Phase A2 PE-transpose vT -> v_nat [s, head_dim] tiles for the AV matmul.
  Phase B  per (head, q-tile 512): for each k-chunk 128 (causal: only
           k <= q-tile end): scores^T = kT_chunk.T @ qT -> PSUM; exp via ACT
           (scale=1/sqrt(128)) -> SBUF f32r; causal mask on diagonal chunks
           via gpsimd affine_select; AV (lhsT=v_nat) and Z (lhsT=ones)
           accumulate in PSUM; normalize oT by 1/Z (partition_broadcast).
  Phase C  out-proj: out[s_tile, n_tile] = sum_h oT_h.T @ wo_h -> DRAM.
"""
import math
from contextlib import ExitStack

import numpy as np

import concourse.bass as bass
import concourse.tile as tile
from concourse import bacc, mybir
from concourse.bass import ds, ts
from concourse.bass_utils import run_bass_kernel_spmd
from concourse.masks import make_identity

P = 128          # partitions / head_dim
SB = 512         # s-block and q-tile width
F32 = mybir.dt.float32
F32R = mybir.dt.float32r

# problem constants
B, S, D = 1, 2048, 4096
H, KVH, HD = 32, 8, 128
NCORES = 8
NQH = H // NCORES      # q heads per core = 4
ROPE_HALF = HD // 2    # 64


def build_attention_kernel(S_=S, D_=D, nqh=NQH, causal=True, loop_reps=None, phases="ABC"):
    """Build the per-core Bass kernel. Returns compiled Bacc object.

    Inputs (per core, DRAM):
      xT      [D_, S_]        f32r  x transposed
      wq      [D_, nqh*128]   f32r  q weights, head-grouped, rope-permuted cols
      wk      [D_, 128]       f32r  rope-permuted cols
      wv      [D_, 128]       f32r
      wo      [nqh*128, D_]   f32r
      cosT    [64, S_]        f32
      sinT    [64, S_]        f32
    Output:
      out     [S_, D_]        f32   partial (this core's heads through wo)
    """
    DC = D_ // P           # d chunks
    SBLK = S_ // SB        # s blocks / q tiles
    SSUB = S_ // P         # s subtiles
    NT = D_ // SB          # out-proj n tiles
    inv_sqrt_hd = 1.0 / math.sqrt(HD)

    nc = bacc.Bacc("TRN2", target_bir_lowering=False, debug=False,
                   num_devices=NCORES)
    xT = nc.dram_tensor("xT", [D_, S_], F32R, kind="ExternalInput").ap()
    wq = nc.dram_tensor("wq", [D_, nqh * P], F32R, kind="ExternalInput").ap()
    wk = nc.dram_tensor("wk", [D_, P], F32R, kind="ExternalInput").ap()
    wv = nc.dram_tensor("wv", [D_, P], F32R, kind="ExternalInput").ap()
    wo = nc.dram_tensor("wo", [nqh * P, D_], F32R, kind="ExternalInput").ap()
    cosT = nc.dram_tensor("cosT", [P, S_], F32, kind="ExternalInput").ap()
    sinT = nc.dram_tensor("sinT", [P, S_], F32, kind="ExternalInput").ap()
    out = nc.dram_tensor("out", [S_, D_], F32, kind="ExternalOutput").ap()

    with tile.TileContext(nc) as tc, ExitStack() as top:
        persist = top.enter_context(tc.tile_pool(name="persist", bufs=1))

        def body():
            with ExitStack() as ctx:
                # ---- persistent SBUF buffers ----
                qT_all = persist.tile([P, nqh, S_], F32R, tag="qT", name="qT_all")
                kT_all = persist.tile([P, S_], F32R, tag="kT", name="kT_all")
                v_nat = persist.tile([P, SSUB, P], F32R, tag="vn", name="v_nat")
                oT_all = persist.tile([P, nqh, S_], F32R, tag="oT", name="oT_all")
                ones32 = persist.tile([P, 1], F32, tag="o32", name="ones32")
                ones_r = persist.tile([P, 1], F32R, tag="or", name="ones_r")
                ident = persist.tile([P, P], F32, tag="idf", name="ident")
                ident_r = persist.tile([P, P], F32R, tag="idr", name="ident_r")

                nc.vector.memset(ones32[:], 1.0)
                nc.vector.tensor_copy(ones_r[:], ones32[:])
                make_identity(nc, ident[:])
                nc.vector.tensor_copy(ident_r[:], ident[:])

                # ---- Phase A: QKV projections + RoPE ----
                with ExitStack() as actx:
                    wpool = actx.enter_context(tc.tile_pool(name="wpool", bufs=1))
                    xpool = actx.enter_context(tc.tile_pool(name="xpool", bufs=3))
                    rtmp = actx.enter_context(tc.tile_pool(name="rtmp", bufs=1))
                    psA = actx.enter_context(
                        tc.tile_pool(name="psA", bufs=1, space="PSUM"))

                    wq_sb = wpool.tile([P, DC, nqh * P], F32R, tag="wq", name="wq_sb")
                    wk_sb = wpool.tile([P, DC, P], F32R, tag="wk", name="wk_sb")
                    wv_sb = wpool.tile([P, DC, P], F32R, tag="wv", name="wv_sb")
                    nc.sync.dma_start(wq_sb[:], wq.rearrange("(o p) m -> p o m", p=P))
                    nc.sync.dma_start(wk_sb[:], wk.rearrange("(o p) m -> p o m", p=P))
                    nc.sync.dma_start(wv_sb[:], wv.rearrange("(o p) m -> p o m", p=P))

                    for sb in range(SBLK):
                        ps_q = [psA.tile([P, SB], F32, tag=f"psq{h}", name=f"ps_q{h}")
                                for h in range(nqh)]
                        ps_k = psA.tile([P, SB], F32, tag="psk", name="ps_k")
                        ps_v = psA.tile([P, SB], F32, tag="psv", name="ps_v")
                        for dc in range(DC):
                            xt = xpool.tile([P, SB], F32R, tag="xt", name="xt")
                            nc.sync.dma_start(
                                xt[:], xT[ds(dc * P, P), ds(sb * SB, SB)])
                            for h in range(nqh):
                                nc.tensor.matmul(
                                    ps_q[h][:], wq_sb[:, dc, ts(h, P)], xt[:],
                                    start=(dc == 0), stop=(dc == DC - 1))
                            nc.tensor.matmul(ps_k[:], wk_sb[:, dc, :], xt[:],
                                             start=(dc == 0), stop=(dc == DC - 1))
                            nc.tensor.matmul(ps_v[:], wv_sb[:, dc, :], xt[:],
                                             start=(dc == 0), stop=(dc == DC - 1))

                        # Stage PSUM->SBUF fast on ACT so the PE can start
                        # the next s-block; RoPE (DVE) then reads the staging.
                        stg = rtmp.tile([P, nqh + 2, SB], F32R, tag="stg",
                                        name="stg")
                        for h in range(nqh):
                            nc.scalar.copy(stg[:, h, :], ps_q[h][:])
                        nc.scalar.copy(stg[:, nqh, :], ps_k[:])
                        nc.scalar.copy(stg[:, nqh + 1, :], ps_v[:])

                        # rope tables for this block: cc = cos||cos,
                        # ss = -sin||sin (host-prepared, full 128 partitions)
                        ssl = ds(sb * SB, SB)
                        cc = rtmp.tile([P, SB], F32, tag="cc", name="cc")
                        ss = rtmp.tile([P, SB], F32, tag="ssb", name="ss")
                        nc.sync.dma_start(cc[:], cosT[:, ssl])
                        nc.sync.dma_start(ss[:], sinT[:, ssl])
                        for h in range(nqh + 1):
                            src = stg[:, h, :]
                            dst = (kT_all[:, ssl] if h == nqh
                                   else qT_all[:, h, ssl])
                            rot = rtmp.tile([P, SB], F32R, tag="rot", name="rot")
                            nc.vector.tensor_copy(
                                rot[0:ROPE_HALF, :], src[ROPE_HALF:P, :])
                            nc.vector.tensor_copy(
                                rot[ROPE_HALF:P, :], src[0:ROPE_HALF, :])
                            tm = rtmp.tile([P, SB], F32, tag="tm", name="tm")
                            nc.vector.tensor_mul(tm[:], rot[:], ss[:])
                            nc.vector.tensor_mul(dst, src, cc[:])
                            nc.vector.tensor_tensor(
                                dst, dst, tm[:], mybir.AluOpType.add)

                        # vT for this block -> transpose to v_nat subtiles
                        with tc.tile_pool(name="psT", bufs=2, space="PSUM") as psT:
                            for st in range(SB // P):
                                ps_t = psT.tile([P, P], F32R, tag="pst", name="ps_t")
                                nc.tensor.transpose(
                                    ps_t[:], stg[:, nqh + 1, ts(st, P)], ident_r[:])
                                nc.vector.tensor_copy(
                                    v_nat[:, sb * (SB // P) + st, :], ps_t[:])

                if phases == "A":
                    # consume qkv so DCE keeps phase A
                    fin = persist.tile([P, SB], F32, tag="fin", name="fin")
                    nc.vector.tensor_mul(fin[:], qT_all[:, 0, 0:SB],
                                         kT_all[:, 0:SB])
                    nc.sync.dma_start(out[ds(0, P), ds(0, SB)], fin[:])
                    return

                # ---- Phase B: attention ----
                with ExitStack() as bctx:
                    wopool = bctx.enter_context(tc.tile_pool(name="wopool", bufs=1))
                    wo_sb = wopool.tile([P, nqh, D_], F32R, tag="wo", name="wo_sb")
                    nc.sync.dma_start(wo_sb[:], wo.rearrange("(o p) m -> p o m", p=P))

                    attn = ExitStack()
                    epool = attn.enter_context(tc.tile_pool(name="epool", bufs=4))
                    zpool = attn.enter_context(tc.tile_pool(name="zpool", bufs=2))
                    psS = attn.enter_context(
                        tc.tile_pool(name="psS", bufs=2, space="PSUM"))
                    psO = attn.enter_context(
                        tc.tile_pool(name="psO", bufs=2, space="PSUM"))

                    for h in range(nqh):
                        for j in range(SBLK):
                            nk = (SB // P) * (j + 1) if causal else SSUB
                            assert nk % 2 == 0
                            qsl = ds(j * SB, SB)
                            ps_o = psO.tile([P, SB], F32, tag="pso", name="ps_o")
                            ps_z = psO.tile([1, SB], F32, tag="psz", name="ps_z")
                            for pc in range(nk // 2):
                                ps_s = psS.tile([P, 2, SB], F32, tag="pss",
                                                name="ps_s")
                                for u in range(2):
                                    nc.tensor.matmul(
                                        ps_s[:, u, :],
                                        kT_all[:, ts(2 * pc + u, P)],
                                        qT_all[:, h, qsl], start=True, stop=True)
                                e2 = epool.tile([P, 2, SB], F32R, tag="et",
                                                name="e2")
                                nc.scalar.activation(
                                    e2[:], ps_s[:],
                                    mybir.ActivationFunctionType.Exp,
                                    scale=inv_sqrt_hd)
                                doff = 2 * pc * P - j * SB
                                if causal and doff > -2 * P:
                                    nc.gpsimd.affine_select(
                                        e2[:], e2[:], pattern=[[-P, 2], [1, SB]],
                                        compare_op=mybir.AluOpType.is_ge,
                                        fill=0.0, base=-doff,
                                        channel_multiplier=-1)
                                for u in range(2):
                                    kc = 2 * pc + u
                                    nc.tensor.matmul(
                                        ps_o[:], v_nat[:, kc, :], e2[:, u, :],
                                        start=(kc == 0), stop=(kc == nk - 1))
                                    nc.tensor.matmul(
                                        ps_z[:], ones_r[:], e2[:, u, :],
                                        start=(kc == 0), stop=(kc == nk - 1))
                            zr = zpool.tile([1, SB], F32R, tag="zr", name="zr")
                            with nc.allow_low_precision(
                                    reason="f32r recip feeds f32r matmul"):
                                nc.vector.reciprocal(zr[:], ps_z[:])
                            zb = zpool.tile([P, SB], F32R, tag="zb", name="zb")
                            nc.gpsimd.partition_broadcast(zb[:], zr[:])
                            nc.vector.tensor_mul(
                                oT_all[:, h, qsl], ps_o[:], zb[:])

                    attn.close()

                    if phases == "AB":
                        fin = persist.tile([P, SB], F32, tag="fin", name="fin")
                        nc.vector.tensor_mul(fin[:], oT_all[:, 0, 0:SB],
                                             oT_all[:, nqh - 1, 0:SB])
                        nc.sync.dma_start(out[ds(0, P), ds(0, SB)], fin[:])
                        return

                    # ---- Phase C: output projection ----
                    with ExitStack() as cctx:
                        opool = cctx.enter_context(
                            tc.tile_pool(name="opool", bufs=4))
                        psC = cctx.enter_context(
                            tc.tile_pool(name="psC", bufs=4, space="PSUM"))
                        for st in range(SSUB):
                            for nt in range(NT):
                                ps_c = psC.tile([P, SB], F32, tag="psc", name="ps_c")
                                for hh in range(nqh):
                                    nc.tensor.matmul(
                                        ps_c[:], oT_all[:, hh, ts(st, P)],
                                        wo_sb[:, hh, ts(nt, SB)],
                                        start=(hh == 0), stop=(hh == nqh - 1))
                                ot = opool.tile([P, SB], F32, tag="ot", name="ot")
                                nc.any.tensor_copy(ot[:], ps_c[:])
                                nc.sync.dma_start(
                                    out[ds(st * P, P), ds(nt * SB, SB)], ot[:])

        if loop_reps is not None:
            with tc.For_i(0, loop_reps, 1):
                body()
        else:
            body()

    nc.compile()
    return nc


_ROPE_PERM = np.concatenate([np.arange(0, HD, 2), np.arange(1, HD, 2)])


def shard_inputs(x, wq, wk, wv, wo, freqs_cos, freqs_sin):
    """Host-side sharding/layout. Returns list of 8 per-core input dicts."""
    x2 = np.ascontiguousarray(np.asarray(x, dtype=np.float32).reshape(S, D))
    xTh = np.ascontiguousarray(x2.T)                       # [D, S]
    cos_h = np.asarray(freqs_cos, np.float32).T                       # [64, S]
    sin_h = np.asarray(freqs_sin, np.float32).T
    cosT = np.ascontiguousarray(np.concatenate([cos_h, cos_h], axis=0))
    sinT = np.ascontiguousarray(np.concatenate([-sin_h, sin_h], axis=0))
    wq = np.asarray(wq, np.float32)
    wk = np.asarray(wk, np.float32)
    wv = np.asarray(wv, np.float32)
    wo = np.asarray(wo, np.float32)
    in_maps = []
    for c in range(NCORES):
        heads = [c + NCORES * r for r in range(NQH)]       # g % KVH == c
        wq_c = np.concatenate(
            [wq[:, g * HD + _ROPE_PERM] for g in heads], axis=1)
        wk_c = wk[:, c * HD + _ROPE_PERM]
        wv_c = wv[:, c * HD:(c + 1) * HD]
        wo_c = np.concatenate([wo[g * HD:(g + 1) * HD, :] for g in heads], axis=0)
        in_maps.append({
            "xT": xTh,
            "wq": np.ascontiguousarray(wq_c),
            "wk": np.ascontiguousarray(wk_c),
            "wv": np.ascontiguousarray(wv_c),
            "wo": np.ascontiguousarray(wo_c),
            "cosT": cosT,
            "sinT": sinT,
        })
    return in_maps


_NC_CACHE = {}


def _get_nc():
    if "nc" not in _NC_CACHE:
        _NC_CACHE["nc"] = build_attention_kernel()
    return _NC_CACHE["nc"]


def kernel(x, wq, wk, wv, wo, freqs_cos, freqs_sin, mask, cache_k, cache_v,
           start_pos):
    assert int(start_pos) == 0, "kernel assumes prefill at start_pos=0"
    in_maps = shard_inputs(x, wq, wk, wv, wo, freqs_cos, freqs_sin)
    nc = _get_nc()
    res = run_bass_kernel_spmd(nc, in_maps, core_ids=list(range(NCORES)))
    acc = np.zeros((S, D), np.float32)
    for c in range(NCORES):
        acc += res.results[c]["out"]
    return acc.reshape(B, S, D)



# revision 2
# speedup vs baseline: 3.1533x; 3.1533x over previous
"""Trainium2 Bass kernel for nn_Attention_7670811590880.

Multi-head attention prefill (B=1, S=2048, D=4096, H=32, KVH=8, HD=128),
tensor-parallel over heads across 8 NeuronCores.

Sharding: query head g uses kv head g % 8 (the reference's _repeat_kv inserts
the repeat axis BEFORE the kv-head axis). Core c takes query heads
{c, c+8, c+16, c+24} and kv head c; wo rows for those heads. Each core
produces a partial [S, D] output; the host sums the 8 partials.

v2 design (bf16 matmuls everywhere; fp32 PSUM accumulation):
  Phase A  per s-block (512): x chunks DMA'd once, 3 passes over the
           contraction dim (q0q1 / q2q3 / kv) with 2 interleaved PSUM
           accumulation groups each; PSUM->SBUF staging on ACT (bf16);
           RoPE on DVE (all-bf16, 2x mode); vT -> v_aug [k,129-col] via
           PE transpose, col 128 = 1.0 (softmax denominator trick).
  Phase B  per (head, q-chunk 128): causal at 128 granularity. Scores
           sT = kT_chunk.T @ qT_chunk -> PSUM f32 in groups of <=8
           chunks; one exp (ACT, scale=1/sqrt(128)) -> bf16; diagonal
           chunk masked by a precomputed 0/1 lower-tri bf16 mask (DVE
           multiply); AV with rhs = v_aug accumulates [q, 128+1] in
           PSUM: col 128 is Z. Normalize on DVE (tensor_scalar_mul by
           reciprocal of Z col), PE-transpose back to oT [hd, q].
  Phase C  out-proj: out[s_tile, n_tile] = sum_h oT_h.T @ wo_h -> DRAM.

Emission is software-pipelined: B rows of s-block j are interleaved with
phase-A passes of s-block j+1 (and B rows of the last block with phase-C
tiles) so ACT exp time hides under PE-bound GEMM work.
"""
import math
from contextlib import ExitStack

import numpy as np
import ml_dtypes

import concourse.bass as bass
import concourse.tile as tile
from concourse import bacc, mybir
from concourse.bass import ds, ts
from concourse.bass_utils import run_bass_kernel_spmd
from concourse.masks import make_identity

P = 128          # partitions / head_dim
SB = 512         # s-block width
F32 = mybir.dt.float32
BF16 = mybir.dt.bfloat16

# problem constants
B, S, D = 1, 2048, 4096
H, KVH, HD = 32, 8, 128
NCORES = 8
NQH = H // NCORES      # q heads per core = 4
ROPE_HALF = HD // 2    # 64


def _interleave(ga, gb, b_per_a):
    """Pull from generator ga once, then ~b_per_a times from gb, until both
    are exhausted. Emission order only; correctness comes from tile deps."""
    err = 0.0
    a_live, b_live = True, True
    while a_live or b_live:
        if a_live:
            try:
                next(ga)
            except StopIteration:
                a_live = False
        err += b_per_a
        n = int(err)
        err -= n
        if not a_live:
            n = 1 << 30
        for _ in range(n):
            if not b_live:
                break
            try:
                next(gb)
            except StopIteration:
                b_live = False
                break


def build_attention_kernel(S_=S, D_=D, nqh=NQH, loop_reps=None):
    """Build the per-core Bass kernel. Returns compiled Bacc object.

    Inputs (per core, DRAM):
      xT      [D_, S_]        bf16  x transposed
      wq      [D_, nqh*128]   bf16  q weights, head-grouped, rope-permuted cols
      wk      [D_, 128]       bf16  rope-permuted cols
      wv      [D_, 128]       bf16
      wo      [nqh*128, D_]   bf16
      cosT    [128, S_]       bf16  cos||cos
      sinT    [128, S_]       bf16  -sin||sin
    Output:
      out     [S_, D_]        f32   partial (this core's heads through wo)
    """
    DC = D_ // P           # d chunks = 32
    SBLK = S_ // SB        # s blocks = 4
    CPB = SB // P          # 128-chunks per block = 4
    SSUB = S_ // P         # s subtiles = 16
    NT = D_ // SB          # out-proj n tiles = 8
    GRP = 8                # k-chunks per score/exp group
    inv_sqrt_hd = 1.0 / math.sqrt(HD)

    nc = bacc.Bacc("TRN2", target_bir_lowering=False, debug=False,
                   num_devices=NCORES)
    xT = nc.dram_tensor("xT", [D_, S_], BF16, kind="ExternalInput").ap()
    wq = nc.dram_tensor("wq", [D_, nqh * P], BF16, kind="ExternalInput").ap()
    wk = nc.dram_tensor("wk", [D_, P], BF16, kind="ExternalInput").ap()
    wv = nc.dram_tensor("wv", [D_, P], BF16, kind="ExternalInput").ap()
    wo = nc.dram_tensor("wo", [nqh * P, D_], BF16, kind="ExternalInput").ap()
    cosT = nc.dram_tensor("cosT", [P, S_], BF16, kind="ExternalInput").ap()
    sinT = nc.dram_tensor("sinT", [P, S_], BF16, kind="ExternalInput").ap()
    out = nc.dram_tensor("out", [S_, D_], F32, kind="ExternalOutput").ap()

    with tile.TileContext(nc) as tc, ExitStack() as top:
        persist = top.enter_context(tc.tile_pool(name="persist", bufs=1))

        def body():
            with ExitStack() as ctx:
                # ---- per-block persistent SBUF (distinct tags => precise
                # block-level dependencies for the software pipeline) ----
                qT = [persist.tile([P, nqh, SB], BF16, tag=f"qT{j}",
                                   name=f"qT{j}") for j in range(SBLK)]
                kT = [persist.tile([P, SB], BF16, tag=f"kT{j}",
                                   name=f"kT{j}") for j in range(SBLK)]
                va = [persist.tile([P, CPB, P + 4], BF16, tag=f"va{j}",
                                   name=f"va{j}") for j in range(SBLK)]
                oT = [persist.tile([P, nqh, SB], BF16, tag=f"oT{j}",
                                   name=f"oT{j}") for j in range(SBLK)]
                ident_f = persist.tile([P, P], F32, tag="idf", name="ident_f")
                ident_b = persist.tile([P, P], BF16, tag="idb", name="ident_b")
                mask_f = persist.tile([P, P], F32, tag="mkf", name="mask_f")
                mask_b = persist.tile([P, P], BF16, tag="mkb", name="mask_b")

                make_identity(nc, ident_f[:])
                nc.vector.tensor_copy(ident_b[:], ident_f[:])
                # lower-triangular-inclusive 0/1 mask: keep q >= k
                # (partition = k, free = q)
                nc.gpsimd.memset(mask_f[:], 1.0)
                nc.gpsimd.affine_select(
                    mask_f[:], mask_f[:], pattern=[[1, P]],
                    compare_op=mybir.AluOpType.is_ge, fill=0.0,
                    base=0, channel_multiplier=-1)
                nc.vector.tensor_copy(mask_b[:], mask_f[:])
                for j in range(SBLK):
                    nc.vector.memset(va[j][:], 1.0)

                attn = ctx.enter_context(ExitStack())
                psT = attn.enter_context(
                    tc.tile_pool(name="psT", bufs=2, space="PSUM"))
                psS = attn.enter_context(
                    tc.tile_pool(name="psS", bufs=1, space="PSUM"))
                psO = attn.enter_context(
                    tc.tile_pool(name="psO", bufs=2, space="PSUM"))
                epool = attn.enter_context(tc.tile_pool(name="epool", bufs=2))
                zpool = attn.enter_context(tc.tile_pool(name="zpool", bufs=2))
                onpool = attn.enter_context(tc.tile_pool(name="onp", bufs=2))

                # ================= phase A =================
                def a_block(j, apool, xpool, stgp, rpool, wq_sb, wk_sb, wv_sb):
                    xb = xpool.tile([P, DC, SB], BF16, tag="xb", name="xb")
                    for dc in range(DC):
                        nc.sync.dma_start(
                            xb[:, dc, :], xT[ds(dc * P, P), ds(j * SB, SB)])

                    def rope(dst, src):
                        cc = rpool.tile([P, SB], BF16, tag="cc", name="cc")
                        ss = rpool.tile([P, SB], BF16, tag="ss", name="ss")
                        nc.sync.dma_start(cc[:], cosT[:, ds(j * SB, SB)])
                        nc.sync.dma_start(ss[:], sinT[:, ds(j * SB, SB)])
                        rot = rpool.tile([P, SB], BF16, tag="rot", name="rot")
                        tm = rpool.tile([P, SB], BF16, tag="tm", name="tm")
                        nc.vector.tensor_copy(rot[0:ROPE_HALF, :],
                                              src[ROPE_HALF:P, :])
                        nc.vector.tensor_copy(rot[ROPE_HALF:P, :],
                                              src[0:ROPE_HALF, :])
                        nc.vector.tensor_mul(tm[:], rot[:], ss[:])
                        nc.vector.tensor_mul(dst, src, cc[:])
                        nc.vector.tensor_tensor(dst, dst, tm[:],
                                                mybir.AluOpType.add)

                    # pass 1: q0,q1  pass 2: q2,q3  pass 3: k,v
                    for pss in range(3):
                        pa = apool.tile([P, SB], F32, tag="psA", name="pa")
                        pb = apool.tile([P, SB], F32, tag="psA", name="pb")
                        for dc in range(DC):
                            if pss < 2:
                                nc.tensor.matmul(
                                    pa[:], wq_sb[:, dc, ts(2 * pss, P)],
                                    xb[:, dc, :],
                                    start=(dc == 0), stop=(dc == DC - 1))
                                nc.tensor.matmul(
                                    pb[:], wq_sb[:, dc, ts(2 * pss + 1, P)],
                                    xb[:, dc, :],
                                    start=(dc == 0), stop=(dc == DC - 1))
                            else:
                                nc.tensor.matmul(
                                    pa[:], wk_sb[:, dc, :], xb[:, dc, :],
                                    start=(dc == 0), stop=(dc == DC - 1))
                                nc.tensor.matmul(
                                    pb[:], wv_sb[:, dc, :], xb[:, dc, :],
                                    start=(dc == 0), stop=(dc == DC - 1))
                            if dc % 8 == 7:
                                yield
                        stg = stgp.tile([P, 2, SB], BF16, tag="stg",
                                        name="stg")
                        nc.scalar.copy(stg[:, 0, :], pa[:])
                        nc.scalar.copy(stg[:, 1, :], pb[:])
                        if pss < 2:
                            rope(qT[j][:, 2 * pss, :], stg[:, 0, :])
                            rope(qT[j][:, 2 * pss + 1, :], stg[:, 1, :])
                        else:
                            rope(kT[j][:, :], stg[:, 0, :])
                            for st in range(CPB):
                                pt = psT.tile([P, P], BF16, tag="psT",
                                              name="pt")
                                nc.tensor.transpose(
                                    pt[:], stg[:, 1, ts(st, P)], ident_b[:])
                                nc.vector.tensor_copy(va[j][:, st, 0:P], pt[:])
                        yield

                # ================= phase B =================
                def b_rows(j):
                    for h in range(nqh):
                        for qcl in range(CPB):
                            qc = CPB * j + qcl
                            nk = qc + 1
                            po = psO.tile([P, P + 1], F32, tag="po", name="po")
                            kc0 = 0
                            while kc0 < nk:
                                glen = min(GRP, nk - kc0)
                                ps_s = psS.tile([P, GRP, P], F32, tag="pss",
                                                name="ps_s")
                                for u in range(glen):
                                    kc = kc0 + u
                                    nc.tensor.matmul(
                                        ps_s[:, u, :],
                                        kT[kc // CPB][:, ts(kc % CPB, P)],
                                        qT[j][:, h, ts(qcl, P)],
                                        start=True, stop=True)
                                e2 = epool.tile([P, GRP, P], BF16, tag="e2",
                                                name="e2")
                                nc.scalar.activation(
                                    e2[:, 0:glen, :], ps_s[:, 0:glen, :],
                                    mybir.ActivationFunctionType.Exp,
                                    scale=inv_sqrt_hd)
                                if kc0 + glen == nk:
                                    # last chunk is the diagonal: causal mask
                                    nc.vector.tensor_mul(
                                        e2[:, glen - 1, :], e2[:, glen - 1, :],
                                        mask_b[:])
                                for u in range(glen):
                                    kc = kc0 + u
                                    nc.tensor.matmul(
                                        po[:, 0:P + 1], e2[:, u, :],
                                        va[kc // CPB][:, kc % CPB, 0:P + 1],
                                        start=(kc == 0), stop=(kc == nk - 1))
                                kc0 += glen
                                yield
                            zr = zpool.tile([P, 1], F32, tag="zr", name="zr")
                            nc.vector.reciprocal(zr[:], po[:, P:P + 1])
                            onrm = onpool.tile([P, P], BF16, tag="on",
                                               name="onrm")
                            nc.vector.tensor_scalar_mul(onrm[:], po[:, 0:P],
                                                        zr[:])
                            pt = psT.tile([P, P], BF16, tag="psT", name="ptB")
                            nc.tensor.transpose(pt[:], onrm[:], ident_b[:])
                            nc.vector.tensor_copy(oT[j][:, h, ts(qcl, P)],
                                                  pt[:])
                            yield

                # ================= phase C =================
                def c_tiles(st_list, psC, copool, wo_sb):
                    for st in st_list:
                        jj, stl = st // CPB, st % CPB
                        for nt in range(NT):
                            pc = psC.tile([P, SB], F32, tag="pc", name="pc")
                            for hh in range(nqh):
                                nc.tensor.matmul(
                                    pc[:], oT[jj][:, hh, ts(stl, P)],
                                    wo_sb[:, hh, ts(nt, SB)],
                                    start=(hh == 0), stop=(hh == nqh - 1))
                            ot = copool.tile([P, SB], F32, tag="ot", name="ot")
                            nc.any.tensor_copy(ot[:], pc[:])
                            nc.sync.dma_start(
                                out[ds(st * P, P), ds(nt * SB, SB)], ot[:])
                            yield

                # ---- segment 1: A blocks pipelined with B rows ----
                with ExitStack() as actx:
                    wpool = actx.enter_context(
                        tc.tile_pool(name="wpool", bufs=1))
                    xpool = actx.enter_context(
                        tc.tile_pool(name="xpool", bufs=2))
                    stgp = actx.enter_context(tc.tile_pool(name="stgp",
                                                           bufs=2))
                    rpool = actx.enter_context(tc.tile_pool(name="rpool",
                                                            bufs=2))
                    apool = actx.enter_context(
                        tc.tile_pool(name="apool", bufs=2, space="PSUM"))

                    wq_sb = wpool.tile([P, DC, nqh * P], BF16, tag="wq",
                                       name="wq_sb")
                    wk_sb = wpool.tile([P, DC, P], BF16, tag="wk",
                                       name="wk_sb")
                    wv_sb = wpool.tile([P, DC, P], BF16, tag="wv",
                                       name="wv_sb")
                    nc.sync.dma_start(wq_sb[:],
                                      wq.rearrange("(o p) m -> p o m", p=P))
                    nc.sync.dma_start(wk_sb[:],
                                      wk.rearrange("(o p) m -> p o m", p=P))
                    nc.sync.dma_start(wv_sb[:],
                                      wv.rearrange("(o p) m -> p o m", p=P))

                    def ab(jj):
                        return a_block(jj, apool, xpool, stgp, rpool,
                                       wq_sb, wk_sb, wv_sb)

                    for _ in ab(0):
                        pass
                    _interleave(ab(1), b_rows(0), 2.0)
                    _interleave(ab(2), b_rows(1), 2.0)
                    _interleave(ab(3), b_rows(2), 3.0)

                # ---- segment 2: C tiles pipelined with last B rows ----
                with ExitStack() as cctx:
                    wopool = cctx.enter_context(
                        tc.tile_pool(name="wopool", bufs=1))
                    copool = cctx.enter_context(
                        tc.tile_pool(name="copool", bufs=4))
                    psC = cctx.enter_context(
                        tc.tile_pool(name="psC", bufs=2, space="PSUM"))
                    wo_sb = wopool.tile([P, nqh, D_], BF16, tag="wo",
                                        name="wo_sb")
                    nc.sync.dma_start(wo_sb[:],
                                      wo.rearrange("(o p) m -> p o m", p=P))
                    _interleave(b_rows(3),
                                c_tiles(range(0, 12), psC, copool, wo_sb),
                                2.0)
                    for _ in c_tiles(range(12, SSUB), psC, copool, wo_sb):
                        pass

        if loop_reps is not None:
            with tc.For_i(0, loop_reps, 1):
                body()
        else:
            body()

    nc.compile()
    return nc


_ROPE_PERM = np.concatenate([np.arange(0, HD, 2), np.arange(1, HD, 2)])


def shard_inputs(x, wq, wk, wv, wo, freqs_cos, freqs_sin):
    """Host-side sharding/layout. Returns list of 8 per-core input dicts."""
    bf = ml_dtypes.bfloat16
    x2 = np.asarray(x, dtype=np.float32).reshape(S, D)
    xTh = np.ascontiguousarray(x2.T.astype(bf))                   # [D, S]
    cos_h = np.asarray(freqs_cos, np.float32).T                   # [64, S]
    sin_h = np.asarray(freqs_sin, np.float32).T
    cosT = np.ascontiguousarray(
        np.concatenate([cos_h, cos_h], axis=0).astype(bf))
    sinT = np.ascontiguousarray(
        np.concatenate([-sin_h, sin_h], axis=0).astype(bf))
    wq = np.asarray(wq, np.float32)
    wk = np.asarray(wk, np.float32)
    wv = np.asarray(wv, np.float32)
    wo = np.asarray(wo, np.float32)
    in_maps = []
    for c in range(NCORES):
        heads = [c + NCORES * r for r in range(NQH)]       # g % KVH == c
        wq_c = np.concatenate(
            [wq[:, g * HD + _ROPE_PERM] for g in heads], axis=1)
        wk_c = wk[:, c * HD + _ROPE_PERM]
        wv_c = wv[:, c * HD:(c + 1) * HD]
        wo_c = np.concatenate([wo[g * HD:(g + 1) * HD, :] for g in heads],
                              axis=0)
        in_maps.append({
            "xT": xTh,
            "wq": np.ascontiguousarray(wq_c.astype(bf)),
            "wk": np.ascontiguousarray(wk_c.astype(bf)),
            "wv": np.ascontiguousarray(wv_c.astype(bf)),
            "wo": np.ascontiguousarray(wo_c.astype(bf)),
            "cosT": cosT,
            "sinT": sinT,
        })
    return in_maps


_NC_CACHE = {}


def _get_nc():
    if "nc" not in _NC_CACHE:
        _NC_CACHE["nc"] = build_attention_kernel()
    return _NC_CACHE["nc"]


def kernel(x, wq, wk, wv, wo, freqs_cos, freqs_sin, mask, cache_k, cache_v,
           start_pos):
    assert int(start_pos) == 0, "kernel assumes prefill at start_pos=0"
    in_maps = shard_inputs(x, wq, wk, wv, wo, freqs_cos, freqs_sin)
    nc = _get_nc()
    res = run_bass_kernel_spmd(nc, in_maps, core_ids=list(range(NCORES)))
    acc = np.zeros((S, D), np.float32)
    for c in range(NCORES):
        acc += res.results[c]["out"]
    return acc.reshape(B, S, D)
